# revision 7
# baseline (speedup 1.0000x reference)
"""DifferentialAttention (B=2, S=2048, D=2048, H=16, KVH=8) on 8 TRN2 NeuronCores.

Sharding: 8 cores = 2 (batch) x 4 (tensor-parallel head groups), as baseline.

v1 rewrite (cost-model-driven):
  - bf16 inputs/weights (host-cast): halves DMA, same matmul rate (1.0 cyc/row)
  - all weights resident in SBUF, loaded once
  - RoPE rotate-half via ONE stream_shuffle (head dims host-permuted so the
    rotate partner sits +/-16 within the same 32-partition quadrant)
  - k head duplication via direct DVE half-adds (no sbuf->sbuf DMAs)
  - scores S1,S2 -> one 2-bank psum tile [128,1024]; ONE merged exp per ki
  - causal mask as -50 bias added by a PE matmul (maskT x identC) into psum,
    so exp gives ~0; no DVE mask multiplies
  - softmax denominators R1,R2 via skinny N=1 matmuls (ET chunks as lhsT,
    ones column rhs) accumulated in a small psum tile -> q-partition layout;
    per-q scalars computed on [128,4] tiles; partition broadcast via PE
    transpose + bf16 outer-product matmuls (cost ~53ns each)
  - eps term: pre = var + 128*eps*R1^2 in q-layout (skinny var matmuls too)
  - rsqrt deferred: ALL pairs' pre collected in [128,64]; ONE Ln + ONE Exp
    at end of phase B (2 act-table loads total instead of 33)
  - o_proj: bf16 otf x resident bf16 wo; psum->sbuf copies on ACT (idle in C);
    bf16 output DMA (host upcasts)
"""

import math
import numpy as np
import ml_dtypes

B, S, D = 2, 2048, 2048
H, KVH = 16, 8
Dh = 64
TP = 4
NCORES = 8
LAYER_IDX = 2
LAMBDA_INIT = 0.8 - 0.6 * math.exp(-0.3 * LAYER_IDX)
EPS = 1e-5
ROPE_THETA = 10000.0

_CACHE = {}


def _build_nc():
    import concourse.bass as bass  # noqa: F401
    import concourse.tile as tile
    from concourse import bacc, mybir

    F32 = mybir.dt.float32
    F32R = mybir.dt.float32r
    BF16 = mybir.dt.bfloat16
    Act = mybir.ActivationFunctionType
    Alu = mybir.AluOpType

    nc = bacc.Bacc("TRN2", target_bir_lowering=False, debug=False)

    xT = nc.dram_tensor("xT", [D, S], BF16, kind="ExternalInput")
    wqT = nc.dram_tensor("wqT", [D, 512], BF16, kind="ExternalInput")
    wkT = nc.dram_tensor("wkT", [D, 256], BF16, kind="ExternalInput")
    wvT = nc.dram_tensor("wvT", [D, 256], BF16, kind="ExternalInput")
    woT = nc.dram_tensor("woT", [512, D], BF16, kind="ExternalInput")
    cosT_d = nc.dram_tensor("cosT", [128, S], F32, kind="ExternalInput")
    ssinT_d = nc.dram_tensor("ssinT", [128, S], F32, kind="ExternalInput")
    maskT_d = nc.dram_tensor("maskT", [128, 128], F32R, kind="ExternalInput")
    identC_d = nc.dram_tensor("identC", [128, 256], F32R, kind="ExternalInput")
    ident_d = nc.dram_tensor("ident", [128, 128], F32, kind="ExternalInput")
    onescol_d = nc.dram_tensor("onescol", [128, 1], BF16, kind="ExternalInput")
    sel4_d = nc.dram_tensor("sel4", [4, 512], BF16, kind="ExternalInput")
    lam_d = nc.dram_tensor("lam", [128, 1], F32, kind="ExternalInput")
    out_d = nc.dram_tensor("out", [S, D], BF16, kind="ExternalOutput")

    KD = D // 128  # 16 contraction tiles
    SHUF = [(i + 16) % 32 for i in range(32)]
    SQ128E = 128.0 * EPS

    with tile.TileContext(nc) as tc:
        with tc.tile_pool(name="const", bufs=1) as constp, \
             tc.tile_pool(name="persist", bufs=1) as persist, \
             tc.tile_pool(name="xtp", bufs=8) as xtp, \
             tc.tile_pool(name="ropet", bufs=3) as rp, \
             tc.tile_pool(name="etp", bufs=3) as etp, \
             tc.tile_pool(name="ebp", bufs=2) as ebp, \
             tc.tile_pool(name="outp", bufs=2) as outp, \
             tc.tile_pool(name="psS", bufs=2, space="PSUM") as psS, \
             tc.tile_pool(name="psOT", bufs=1, space="PSUM") as psOT, \
             tc.tile_pool(name="psSm", bufs=2, space="PSUM") as psSm:

            cosT = constp.tile([128, S], F32, tag="cos")
            ssinT = constp.tile([128, S], F32, tag="ssin")
            maskT = constp.tile([128, 128], F32R, tag="mask")
            identC = constp.tile([128, 256], F32R, tag="idc")
            ident = constp.tile([128, 128], F32, tag="id")
            onescol = constp.tile([128, 1], BF16, tag="onc")
            sel4 = constp.tile([4, 512], BF16, tag="sel4")
            lam = constp.tile([128, 1], F32, tag="lam")

            wq_sb = persist.tile([128, KD * 512], BF16, tag="wq")
            wk_sb = persist.tile([128, KD * 256], BF16, tag="wk")
            wv_sb = persist.tile([128, KD * 256], BF16, tag="wv")
            wo_sb = persist.tile([128, 4 * 2048], BF16, tag="wo")

            qT_sb = [persist.tile([128, S], BF16, tag=f"qT{m}", name=f"qT{m}")
                     for m in range(4)]
            kTd = [persist.tile([128, S], BF16, tag=f"kTd{p}", name=f"kTd{p}")
                   for p in range(4)]
            v_sb = [persist.tile([128, 256], BF16, tag=f"v{ms}", name=f"v{ms}")
                    for ms in range(16)]
            otf = [persist.tile([128, S], BF16, tag=f"otf{p}", name=f"otf{p}")
                   for p in range(4)]
            pre_all = persist.tile([128, 64], F32, tag="pre")
            sf_all = persist.tile([128, 64], F32, tag="sf")

            wqv = wq_sb[:].rearrange("p (kd n) -> p kd n", kd=KD)
            wkv = wk_sb[:].rearrange("p (kd n) -> p kd n", kd=KD)
            wvv = wv_sb[:].rearrange("p (kd n) -> p kd n", kd=KD)
            wov = wo_sb[:].rearrange("p (kc n) -> p kc n", kc=4)

            xt_tiles = {}

            def load_x_one(sh, kp):
                c0 = 512 * sh
                t = xtp.tile([128, 2048], BF16, tag="xt", name=f"xt{sh}_{kp}")
                nc.sync.dma_start(
                    out=t[:].rearrange("p (four n) -> p four n", four=4),
                    in_=xT[kp * 512:kp * 512 + 512, c0:c0 + 512]
                        .rearrange("(four p) n -> p four n", four=4),
                )
                xt_tiles[sh, kp] = t

            def load_x(sh):
                for kp in range(4):
                    load_x_one(sh, kp)

            def xt_rhs(sh, kd):
                return xt_tiles[sh, kd // 4][:, (kd % 4) * 512:(kd % 4) * 512 + 512]

            def load_wq_quarter(qt):
                nc.sync.dma_start(
                    out=wqv[:, qt * 4:(qt + 1) * 4, :],
                    in_=wqT[qt * 512:(qt + 1) * 512, :]
                        .rearrange("(kd p) n -> p kd n", kd=4),
                )

            # startup-latency-aware DMA order
            load_x_one(0, 0)
            load_wq_quarter(0)
            load_x_one(0, 1)
            load_wq_quarter(1)
            nc.sync.dma_start(out=cosT[:], in_=cosT_d[:])
            load_x_one(0, 2)
            load_wq_quarter(2)
            nc.sync.dma_start(out=ssinT[:], in_=ssinT_d[:])
            load_x_one(0, 3)
            load_wq_quarter(3)
            nc.sync.dma_start(
                out=wkv[:], in_=wkT[:].rearrange("(kd p) n -> p kd n", kd=KD))
            nc.sync.dma_start(
                out=wvv[:], in_=wvT[:].rearrange("(kd p) n -> p kd n", kd=KD))
            load_x(1)
            nc.sync.dma_start(out=maskT[:], in_=maskT_d[:])
            nc.sync.dma_start(out=identC[:], in_=identC_d[:])
            nc.sync.dma_start(out=ident[:], in_=ident_d[:])
            nc.sync.dma_start(out=onescol[:], in_=onescol_d[:])
            nc.sync.dma_start(out=sel4[:], in_=sel4_d[:])
            nc.sync.dma_start(out=lam[:], in_=lam_d[:])
            nc.sync.dma_start(
                out=wov[:], in_=woT[:].rearrange("(kc p) n -> p kc n", kc=4))

            # ---------------- phase A chunk: projections + RoPE for one sh ----
            def proj_sh(sh):
                c0 = 512 * sh
                csl = slice(c0, c0 + 512)
                if sh + 2 < 4:
                    load_x(sh + 2)
                for m in range(4):
                    qps = psS.tile([128, 1024], F32, tag="s", name=f"qps{m}")
                    for kd in range(KD):
                        nc.tensor.matmul(
                            qps[:, 0:512], wqv[:, kd, m * 128:m * 128 + 128],
                            xt_rhs(sh, kd),
                            start=(kd == 0), stop=(kd == KD - 1),
                        )
                    qsw = rp.tile([128, 512], F32, tag="sw", name="qsw")
                    nc.vector.stream_shuffle(qsw[:], qps[:, 0:512], SHUF)
                    qc = rp.tile([128, 512], F32, tag="qc", name="qc")
                    nc.vector.tensor_mul(qc[:], qps[:, 0:512], cosT[:, csl])
                    nc.vector.tensor_mul(qsw[:], qsw[:], ssinT[:, csl])
                    nc.vector.tensor_add(qT_sb[m][:, csl], qc[:], qsw[:])
                for m in range(2):
                    kps = psS.tile([128, 1024], F32, tag="s", name=f"kps{m}")
                    for kd in range(KD):
                        nc.tensor.matmul(
                            kps[:, 0:512], wkv[:, kd, m * 128:m * 128 + 128],
                            xt_rhs(sh, kd),
                            start=(kd == 0), stop=(kd == KD - 1),
                        )
                    ksw = rp.tile([128, 512], F32, tag="sw", name="ksw")
                    nc.vector.stream_shuffle(ksw[:], kps[:, 0:512], SHUF)
                    kc = rp.tile([128, 512], F32, tag="qc", name="kc")
                    nc.vector.tensor_mul(kc[:], kps[:, 0:512], cosT[:, csl])
                    nc.vector.tensor_mul(ksw[:], ksw[:], ssinT[:, csl])
                    for e in range(2):
                        esl = slice(e * 64, e * 64 + 64)
                        for hf in range(2):
                            nc.vector.tensor_add(
                                kTd[2 * m + e][hf * 64:hf * 64 + 64, csl],
                                kc[esl, :], ksw[esl, :])
                for ms in range(4):
                    vps = psS.tile([128, 1024], F32, tag="s", name=f"vps{ms}")
                    for kd in range(KD):
                        nc.tensor.matmul(
                            vps[:, 0:256],
                            xt_rhs(sh, kd)[:, ms * 128:ms * 128 + 128],
                            wvv[:, kd, :],
                            start=(kd == 0), stop=(kd == KD - 1),
                        )
                    nc.scalar.copy(v_sb[sh * 4 + ms][:], vps[:, 0:256])

            # ---------------- phase B: one pair's ki loop + early epilogue ----
            def emit_ki_loop(qi, p):
                vh = p // 2
                q0 = 512 * qi
                kis = list(range(4 * qi, 4 * qi + 4)) + list(range(4 * qi))
                OT = psOT.tile([128, 1024], F32, tag="ot", name="OT")
                OTv = OT[:].rearrange("p (two n) -> p two n", two=2)
                smalls = psSm.tile([128, 512], F32, tag="sm", name="smalls")
                nc.vector.memset(smalls[:, 0:128], 0.0)
                nki = len(kis)
                for idx, ki in enumerate(kis):
                    j = ki - 4 * qi
                    diag = j >= 0
                    vc = 128 * j if diag and j > 0 else 0
                    ksl = slice(ki * 128, ki * 128 + 128)
                    S12 = psS.tile([128, 1024], F32, tag="s", name="S12")
                    S12v = S12[:].rearrange("p (two n) -> p two n", two=2)
                    for h in range(2):
                        hsl = slice(h * 64, h * 64 + 64)
                        nc.tensor.matmul(
                            S12v[:, h, vc:512],
                            kTd[p][hsl, ksl],
                            qT_sb[p][hsl, q0 + vc:q0 + 512],
                            start=True, stop=not diag,
                        )
                    if diag:
                        nc.tensor.matmul(
                            S12v[:, :, vc:vc + 128], maskT[:], identC[:],
                            start=False, stop=True,
                        )
                    ET = etp.tile([128, 1024], BF16, tag="e", name="ET")
                    ETv = ET[:].rearrange("p (two n) -> p two n", two=2)
                    nc.scalar.activation(ETv[:, :, vc:512], S12v[:, :, vc:512],
                                         Act.Exp)
                    vt = v_sb[ki][:, vh * 128:vh * 128 + 128]
                    st = idx == 0
                    sp = idx == nki - 1
                    for h in range(2):
                        nc.tensor.matmul(OTv[:, h, vc:512], vt,
                                         ETv[:, h, vc:512], start=st, stop=sp)
                    for h in range(2):
                        for c in range(4):
                            if diag and c < j:
                                continue
                            col = h * 64 + c * 16 + idx
                            nc.tensor.matmul(
                                smalls[:, col:col + 1],
                                ETv[:, h, c * 128:c * 128 + 128],
                                onescol[:],
                                start=True, stop=True,
                            )
                OTs = ebp.tile([128, 1024], F32, tag="ots", name="OTs")
                nc.vector.tensor_copy(OTs[:, 0:512], OTv[:, 0, :])
                nc.vector.tensor_copy(OTs[:, 512:1024], OTv[:, 1, :])
                Rred = ebp.tile([128, 8], F32, tag="rred", name="Rred")
                nc.vector.tensor_reduce(
                    Rred[:],
                    smalls[:, 0:128].rearrange("p (hc k) -> p hc k", k=16),
                    mybir.AxisListType.X, Alu.add)
                rcp2 = ebp.tile([128, 4], F32, tag="rcp", name="rcp2")
                nc.vector.reciprocal(rcp2[:], Rred[:, 4:8])
                m_q = ebp.tile([128, 4], F32, tag="mq", name="m_q")
                nc.vector.scalar_tensor_tensor(
                    m_q[:], Rred[:, 0:4], lam[:, 0:1], rcp2[:],
                    Alu.mult, Alu.mult)
                t2 = ebp.tile([128, 4], F32, tag="t2", name="t2")
                nc.vector.scalar_tensor_tensor(
                    t2[:], Rred[:, 0:4], SQ128E, Rred[:, 0:4],
                    Alu.mult, Alu.mult)
                return (qi, p, OTs, smalls, m_q, t2)

            def emit_late_epilogue(ctx):
                qi, p, OTs, smalls, m_q, t2 = ctx
                q0 = 512 * qi
                nc.tensor.transpose(smalls[0:4, 256:384], m_q[:], ident[:])
                mrow = ebp.tile([4, 128], BF16, tag="mrow", name="mrow")
                nc.vector.tensor_copy(mrow[:], smalls[0:4, 256:384])
                m_b = psS.tile([128, 1024], F32, tag="s", name="m_b")
                for c in range(4):
                    nc.tensor.matmul(m_b[:, c * 128:c * 128 + 128],
                                     sel4[:, c * 128:c * 128 + 128], mrow[:],
                                     start=True, stop=True)
                tt = ebp.tile([128, 512], F32, tag="tt", name="tt")
                nc.vector.tensor_mul(tt[:], OTs[:, 512:1024], m_b[:, 0:512])
                nc.vector.tensor_sub(otf[p][:, q0:q0 + 512], OTs[:, 0:512], tt[:])
                sq = ebp.tile([128, 512], BF16, tag="sq", name="sq")
                nc.vector.tensor_mul(sq[:], otf[p][:, q0:q0 + 512],
                                     otf[p][:, q0:q0 + 512])
                for c in range(4):
                    nc.tensor.matmul(smalls[:, 128 + c:129 + c],
                                     sq[:, c * 128:c * 128 + 128],
                                     onescol[:], start=True, stop=True)
                off = (qi * 4 + p) * 4
                nc.vector.tensor_add(pre_all[:, off:off + 4],
                                     smalls[:, 128:132], t2[:])

            pending = [None]

            def attn_group(qi):
                for p in range(4):
                    ctx = emit_ki_loop(qi, p)
                    if pending[0] is not None:
                        emit_late_epilogue(pending[0])
                    pending[0] = ctx

            def flush_pending():
                if pending[0] is not None:
                    emit_late_epilogue(pending[0])
                    pending[0] = None

            # ---------------- B.5 for one qi group: sf + scale otf ------------
            def bfive_group(qi):
                goff = qi * 16
                lnp = ebp.tile([128, 16], F32, tag="lnp", name="lnp")
                nc.scalar.activation(lnp[:], pre_all[:, goff:goff + 16],
                                     Act.Ln, scale=1.0 / 128.0)
                nc.scalar.activation(sf_all[:, goff:goff + 16], lnp[:],
                                     Act.Exp, scale=-0.5)
                q0 = 512 * qi
                for p in range(4):
                    off = goff + p * 4
                    sm2 = psSm.tile([128, 512], F32, tag="sm", name="sm2")
                    nc.tensor.transpose(sm2[0:4, 256:384],
                                        sf_all[:, off:off + 4], ident[:])
                    sfrow = ebp.tile([4, 128], BF16, tag="mrow", name="sfrow")
                    nc.vector.tensor_copy(sfrow[:], sm2[0:4, 256:384])
                    sf_b = psS.tile([128, 1024], F32, tag="s", name="sf_b")
                    for c in range(4):
                        nc.tensor.matmul(sf_b[:, c * 128:c * 128 + 128],
                                         sel4[:, c * 128:c * 128 + 128], sfrow[:],
                                         start=True, stop=True)
                    nc.vector.tensor_mul(otf[p][:, q0:q0 + 512],
                                         otf[p][:, q0:q0 + 512], sf_b[:, 0:512])

            # ---------------- phase C chunk: o_proj for 4 m tiles ------------
            def oproj_group(g):
                for m in range(4 * g, 4 * g + 4):
                    osb = outp.tile([128, 2048], BF16, tag="ob", name="osb")
                    for n in range(4):
                        ps = psS.tile([128, 1024], F32, tag="s", name="pc")
                        for kc in range(4):
                            nc.tensor.matmul(
                                ps[:, 0:512],
                                otf[kc][:, m * 128:m * 128 + 128],
                                wov[:, kc, n * 512:n * 512 + 512],
                                start=(kc == 0), stop=(kc == 3),
                            )
                        nc.scalar.copy(osb[:, n * 512:n * 512 + 512], ps[:, 0:512])
                    nc.sync.dma_start(out=out_d[m * 128:m * 128 + 128, :], in_=osb[:])

            # ---------------- interleaved schedule ----------------
            proj_sh(0)
            proj_sh(1)
            attn_group(0)
            proj_sh(2)
            attn_group(1)
            flush_pending()
            bfive_group(0)
            oproj_group(0)
            proj_sh(3)
            attn_group(2)
            flush_pending()
            bfive_group(1)
            oproj_group(1)
            attn_group(3)
            flush_pending()
            bfive_group(2)
            oproj_group(2)
            bfive_group(3)
            oproj_group(3)

    nc.compile()
    return nc


def _perm64():
    return np.array(list(range(0, 16)) + list(range(32, 48)) +
                    list(range(16, 32)) + list(range(48, 64)))


def _host_tables():
    p64 = _perm64()
    inv = ROPE_THETA ** (-np.arange(Dh, dtype=np.float64) / Dh)
    pos = np.arange(S, dtype=np.float64)
    fr = pos[:, None] * inv[None, :]              # [S, 64]
    cos = np.cos(fr).astype(np.float32)           # [S, 64]
    sin = np.sin(fr).astype(np.float32)
    d = p64[np.arange(128) % 64]
    cosT = np.ascontiguousarray(cos[:, d].T)      # [128, S]
    sgn = np.where(d < 32, -1.0, 1.0).astype(np.float32)
    ssinT = np.ascontiguousarray(sin[:, d].T * sgn[:, None])
    maskT = np.triu(np.full((128, 128), -50.0, np.float32), 1)
    identC = np.ascontiguousarray(
        np.concatenate([np.eye(128, dtype=np.float32)] * 2, axis=1))
    ident = np.eye(128, dtype=np.float32)
    onescol = np.ones((128, 1), np.float32).astype(ml_dtypes.bfloat16)
    sel4 = np.zeros((4, 512), np.float32)
    for c in range(4):
        sel4[c, c * 128:(c + 1) * 128] = 1.0
    sel4 = sel4.astype(ml_dtypes.bfloat16)
    return cosT, ssinT, maskT, identC, ident, onescol, sel4


def kernel(hidden_states, Wq, Wk, Wv, Wo,
           lambda_q1, lambda_k1, lambda_q2, lambda_k2, subln_weight):
    from concourse.bass_utils import run_bass_kernel_spmd

    if "nc" not in _CACHE:
        _CACHE["nc"] = _build_nc()
        _CACHE["tables"] = _host_tables()
    nc = _CACHE["nc"]
    cosT, ssinT, maskT, identC, ident, onescol, sel4 = _CACHE["tables"]

    f32 = np.float32
    bf16 = ml_dtypes.bfloat16
    hs = np.asarray(hidden_states, f32)
    Wq = np.asarray(Wq, f32)
    Wk = np.asarray(Wk, f32)
    Wv = np.asarray(Wv, f32)
    Wo = np.asarray(Wo, f32)
    subln = np.asarray(subln_weight, f32)

    lam1 = np.exp(np.sum(np.asarray(lambda_q1, f32) * np.asarray(lambda_k1, f32),
                         dtype=f32))
    lam2 = np.exp(np.sum(np.asarray(lambda_q2, f32) * np.asarray(lambda_k2, f32),
                         dtype=f32))
    lam_full = f32(lam1 - lam2 + LAMBDA_INIT)
    lam_arr = np.full((128, 1), lam_full, f32)

    scale = f32(Dh ** -0.5)
    wprime = (np.tile(subln, H) * f32(1.0 - LAMBDA_INIT)).astype(f32)  # [2048]
    WoS = Wo * wprime[None, :]

    p64 = _perm64()
    qperm = (np.repeat(np.arange(8) * 64, 64) + np.tile(p64, 8))
    kperm = (np.repeat(np.arange(4) * 64, 64) + np.tile(p64, 4))

    in_maps = []
    for c in range(NCORES):
        b, r = c // TP, c % TP
        wq_h = np.ascontiguousarray(
            (Wq[512 * r:512 * r + 512, :] * scale).T[:, qperm]).astype(bf16)
        wk_h = np.ascontiguousarray(
            Wk[256 * r:256 * r + 256, :].T[:, kperm]).astype(bf16)
        wv_h = np.ascontiguousarray(Wv[256 * r:256 * r + 256, :].T).astype(bf16)
        wo_h = np.ascontiguousarray(WoS[:, 512 * r:512 * r + 512].T).astype(bf16)
        in_maps.append({
            "xT": np.ascontiguousarray(hs[b].T).astype(bf16),
            "wqT": wq_h, "wkT": wk_h, "wvT": wv_h, "woT": wo_h,
            "cosT": cosT, "ssinT": ssinT, "maskT": maskT, "identC": identC,
            "ident": ident, "onescol": onescol, "sel4": sel4,
            "lam": lam_arr,
        })

    res = run_bass_kernel_spmd(nc, in_maps, core_ids=list(range(NCORES)))
    out = np.zeros((B, S, D), f32)
    for c in range(NCORES):
        out[c // TP] += np.asarray(res.results[c]["out"]).astype(f32)
    return out
